# revision 13
# baseline (speedup 1.0000x reference)
"""MinLSTM layer on 8 Trainium2 NeuronCores.

Math (equivalent to the log-space reference, done in linear space):
    f_pre = x @ W_f.T + b_f ; i_pre = x @ W_i.T + b_i ; h_pre = x @ W_h.T + b_h
    sf = sigmoid(f_pre) ; si = sigmoid(i_pre)
    f = sf / (sf + si)                       # normalized forget gate
    i = 1 - f                                # = si / (sf + si)
    g = max(sigmoid(h_pre), h_pre + 0.5)     # == exp(log_g), exactly
    h_t = f_t * h_{t-1} + i_t * g_t,  h_0 = 1
The gates satisfy f in (0,1), g > 0, so h stays in a tame range and the
recurrence is numerically stable in fp32. Matmul operands are bf16 (max rel
err vs the fp32 log-space reference ~7e-3, well under the 2e-2 gate); PSUM
accumulation and all post-matmul arithmetic are fp32.

Sharding: 8 cores = batch(4) x hidden-halves(2). Core c handles batch b=c//2,
hidden slice [(c%2)*512, (c%2+1)*512). No cross-core communication; the scan
runs along T inside each core via the DVE TensorTensorScan instruction
(state = f*state - mv per step, mv = (f-1)*g = -i*g).

Device layout: gates computed as [h_part, t_free] via out = W_sliceT.T @ xT;
host pre-transposes x and W (numpy, to bf16) and re-transposes the [512, 4096]
per-core output back to [T, Dh]. Matmuls run in 512-wide t-chunks (one PSUM
bank); elementwise+scan run in up-to-1024-wide super-chunks.

Scheduling notes:
- bf16 operands halve HBM traffic (x 8MB, W 3MB per core) so the DMA stream
  (~390 GB/s) always stays ahead of the PE's gate-major J0 burn rate
  (~300 GB/s) -- no mid-stream PE stall, no HAM re-throttle.
- Only 2 warmup matmuls: real matmuls start as soon as the first (x, W_f)
  k-slices land and run at the cold 1.2 GHz rate until the HAM window
  (~3.4us) elapses; running real work cold beats burning the window on
  scratch warmups.
- In J1+ units, the two 512-wide halves of a (gate, k) pair run back-to-back
  on the same loaded weight tile (walrus can skip the second LDWEIGHTS).
- Elementwise work is split across three engines so no single engine's queue
  sits on the critical path: ACT does the 3 sigmoids (PSUM->SBUF), Pool does
  s = sf+si and f = sf*r, DVE does g = max(h_pre+.5, sg) (PSUM read),
  r = 1/s, mv = (f-1)*g, and the scan.
- The final chunk's last h-tile runs its elementwise/scan at 256-wide grain
  to shorten the serial chain after the last matmul.
"""

import sys

for _p in ("/opt/trn_rl_repo",):
    if _p not in sys.path:
        sys.path.append(_p)

import numpy as np
from ml_dtypes import bfloat16

import concourse.bass as bass
import concourse.tile as tile
from concourse import bacc, mybir
from concourse.bass_utils import run_bass_kernel_spmd

B, T, DIN, DH = 4, 4096, 1024, 1024
N_CORES = 8
HSH = DH // 2          # 512 hidden channels per core
P = 128                # partitions
KT = DIN // P          # 8 contraction tiles
NT = 512               # matmul t-chunk (free dim, one PSUM bank)
IT = HSH // P          # 4 h-tiles per core
# elementwise/scan super-chunks (start, length); tail chunks smaller to
# shrink the end-of-kernel drain
CHUNKS = [(0, 1024), (1024, 1024), (2048, 1024), (3072, 512),
          (3584, 256), (3840, 256)]

MM_DT = mybir.dt.bfloat16
NP_MM_DT = bfloat16

_COMPILED = None


def _build():
    AF = mybir.ActivationFunctionType
    OP = mybir.AluOpType
    f32 = mybir.dt.float32

    nc = bacc.Bacc("TRN2", target_bir_lowering=False, debug=False)

    xT = nc.dram_tensor("xT", [DIN, T], MM_DT, kind="ExternalInput").ap()
    wd = {g: nc.dram_tensor(f"w{g}", [DIN, HSH], MM_DT, kind="ExternalInput").ap()
          for g in ("f", "i", "h")}
    # packed per-partition scalars: [b_f | b_i | b_h | b_h+0.5], each (128, IT)
    biases = nc.dram_tensor("biases", [P, 4 * IT], f32, kind="ExternalInput").ap()
    out = nc.dram_tensor("out", [HSH, T], f32, kind="ExternalOutput").ap()

    # DRAM views: (KT*P, n) -> [p, k, n]
    xT_v = xT.rearrange("(k p) t -> p k t", p=P)
    w_v = {g: w.rearrange("(k p) h -> p k h", p=P) for g, w in wd.items()}

    with tile.TileContext(nc) as tc:
        with (
            tc.tile_pool(name="wpool", bufs=1) as wpool,
            tc.tile_pool(name="bpool", bufs=1) as bpool,
            tc.tile_pool(name="xpool", bufs=32) as xpool,
            tc.tile_pool(name="psum", bufs=8, space="PSUM") as pspool,
            tc.tile_pool(name="work", bufs=4) as work,
            tc.tile_pool(name="hpool", bufs=6) as hpool,
        ):
            bias_t = bpool.tile([P, 4 * IT], f32, tag="bias")

            # per-k weight tiles, resident all kernel
            wt = {g: [wpool.tile([P, HSH], MM_DT, tag=f"w{g}{k}", name=f"w{g}{k}_t")
                      for k in range(KT)] for g in ("f", "i", "h")}

            def dma_w(g):
                for k in range(KT):
                    nc.sync.dma_start(out=wt[g][k][:], in_=w_v[g][:, k, :])

            def x_ktiles(t0, nt=NT):
                """One [P, nt] tile per contraction slice k of a t-chunk."""
                xs = []
                for k in range(KT):
                    xk = xpool.tile([P, nt], MM_DT, tag="xk", name="xk_t")
                    nc.sync.dma_start(out=xk[:], in_=xT_v[:, k, t0:t0 + nt])
                    xs.append(xk)
                return xs

            def bias_ap(kind, i):
                return bias_t[:, kind * IT + i:kind * IT + i + 1]

            def chain(i, sf, si, gt, J, t0, ne, grain=None, pool_ok=True):
                """Normalize gates, build -i*g, scan, and store chunk.

                sf/si are the raw sigmoids; gt already holds g (from the
                per-half PSUM stage). grain < ne splits the work into
                sub-chunks so the last chunk's serial chain is short.
                pool_ok moves the add/mul to the Pool engine (2x slower but
                otherwise idle) to keep the DVE queue from backlogging; the
                final units keep everything on the DVE for minimum latency.
                """
                grain = grain or ne
                adder = nc.gpsimd if pool_ok else nc.vector
                for c0 in range(0, ne, grain):
                    cs = slice(c0, c0 + grain)
                    adder.tensor_add(si[:, cs], sf[:, cs], si[:, cs])
                    r = work.tile([P, grain], f32, tag="s", name="r_t")
                    nc.vector.reciprocal_approx_fast(out=r[:], in_=si[:, cs])
                    adder.tensor_mul(sf[:, cs], sf[:, cs], r[:])          # f
                    nc.vector.scalar_tensor_tensor(                # mv=(f-1)*g
                        gt[:, cs], sf[:, cs], 1.0, gt[:, cs],
                        op0=OP.subtract, op1=OP.mult)
                    hc = hpool.tile([P, grain], f32, tag="h", name=f"h{i}_t")
                    init = 1.0 if J == 0 and c0 == 0 else hprev[i][:, -1:]
                    nc.vector.tensor_tensor_scan(
                        hc[:], sf[:, cs], gt[:, cs], init,
                        op0=OP.mult, op1=OP.subtract)
                    hprev[i] = hc
                    nc.sync.dma_start(
                        out=out[i * P:(i + 1) * P, t0 + c0:t0 + c0 + grain],
                        in_=hc[:])

            hprev = [None] * IT
            hsls = [slice(i * P, (i + 1) * P) for i in range(IT)]

            # Warmup matmuls on a zeroed scratch tile open the PE's HAM
            # activity window while the first (x, W_f) DMAs land; the memset
            # runs on the Pool queue, whose preamble finishes first, so the
            # PE goes busy the moment its own preamble ends. Six warmups
            # bridge until real data arrives (~8.3us) without a gap that
            # would reset the HAM window; the clock then reaches 2.4 GHz
            # ~3.4us after the first warmup instead of mid-stream.
            scratch = bpool.tile([P, NT], MM_DT, tag="scratch")
            nc.gpsimd.memset(scratch[:].bitcast(mybir.dt.uint16), 0)
            pswarm = pspool.tile([P, NT], f32, tag="ps", name="pswarm_t")
            for _ in range(6):
                nc.tensor.matmul(pswarm[:], lhsT=scratch[:, :P], rhs=scratch[:],
                                 start=True, stop=True)

            # ---- J0: gate-major, k-outer; PE chases the input DMA stream ----
            t0, ne = CHUNKS[0]
            nhalf = ne // NT
            # priority order: (x_h0[k], W_f[k]) pairs, x_h1, W_i, W_h
            xcs = [[xpool.tile([P, NT], MM_DT, tag="xk", name="xk_t")
                    for _ in range(KT)] for _ in range(nhalf)]
            for k in range(KT):
                nc.sync.dma_start(out=xcs[0][k][:], in_=xT_v[:, k, t0:t0 + NT])
                nc.sync.dma_start(out=wt["f"][k][:], in_=w_v["f"][:, k, :])
                if k == 0:
                    # bias is tiny and first needed by the ACTs at ~12us;
                    # issue it after the first matmul's dependencies
                    nc.sync.dma_start(out=bias_t[:], in_=biases[:])
            for h in range(1, nhalf):
                th = t0 + h * NT
                for k in range(KT):
                    nc.sync.dma_start(out=xcs[h][k][:], in_=xT_v[:, k, th:th + NT])
            dma_w("i")
            dma_w("h")

            sf = [work.tile([P, ne], f32, tag="sf", name="sf_t") for _ in range(IT)]
            si = [work.tile([P, ne], f32, tag="si", name="si_t") for _ in range(IT)]
            sg = [work.tile([P, ne], f32, tag="sg", name="sg_t") for _ in range(IT)]
            gt = [work.tile([P, ne], f32, tag="gt", name="gt_t") for _ in range(IT)]
            for gate, dsts, bk in (("f", sf, 0), ("i", si, 1), ("h", sg, 2)):
                for half in range(nhalf):
                    esl = slice(half * NT, (half + 1) * NT)
                    psts = [pspool.tile([P, NT], f32, tag="ps", name="ps_t")
                            for _ in range(IT)]
                    for k in range(KT):
                        for pst, hsl in zip(psts, hsls):
                            nc.tensor.matmul(
                                pst[:], lhsT=wt[gate][k][:, hsl],
                                rhs=xcs[half][k][:],
                                start=(k == 0), stop=(k == KT - 1))
                    for i in range(IT):
                        nc.scalar.activation(dsts[i][:, esl], psts[i][:], AF.Sigmoid,
                                             bias=bias_ap(bk, i), scale=1.0)
                        if gate == "h":
                            nc.vector.scalar_tensor_tensor(
                                gt[i][:, esl], psts[i][:], bias_ap(3, i),
                                sg[i][:, esl], op0=OP.add, op1=OP.max)
            for i in range(IT):
                chain(i, sf[i], si[i], gt[i], 0, t0, ne)

            # ---- J1+: h-tile-major units; strips share each weight tile ----
            for J, (t0, ne) in enumerate(CHUNKS[1:], start=1):
                st = min(NT, ne)            # matmul strip width
                nstrip = ne // st
                xcs = [x_ktiles(t0 + h * st, st) for h in range(nstrip)]
                for i in range(IT):
                    hsl = hsls[i]
                    sf = work.tile([P, ne], f32, tag="sf", name="sf_t")
                    si = work.tile([P, ne], f32, tag="si", name="si_t")
                    sg = work.tile([P, ne], f32, tag="sg", name="sg_t")
                    gt = work.tile([P, ne], f32, tag="gt", name="gt_t")
                    for gate, dst, bk in (("f", sf, 0), ("i", si, 1),
                                          ("h", sg, 2)):
                        psts = [pspool.tile([P, st], f32, tag="ps", name="ps_t")
                                for _ in range(nstrip)]
                        for k in range(KT):
                            for half in range(nstrip):
                                nc.tensor.matmul(
                                    psts[half][:], lhsT=wt[gate][k][:, hsl],
                                    rhs=xcs[half][k][:],
                                    start=(k == 0), stop=(k == KT - 1))
                        for half in range(nstrip):
                            esl = slice(half * st, (half + 1) * st)
                            nc.scalar.activation(dst[:, esl], psts[half][:],
                                                 AF.Sigmoid, bias=bias_ap(bk, i),
                                                 scale=1.0)
                            if gate == "h":
                                nc.vector.scalar_tensor_tensor(
                                    gt[:, esl], psts[half][:], bias_ap(3, i),
                                    sg[:, esl], op0=OP.add, op1=OP.max)
                    chain(i, sf, si, gt, J, t0, ne,
                          grain=128 if (J == len(CHUNKS) - 1 and i == IT - 1)
                          else None, pool_ok=J <= 3)

    nc.compile()
    return nc


def _in_maps(x, W_f, b_f, W_i, b_i, W_h, b_h):
    x = np.asarray(x, np.float32)
    wT = {g: np.ascontiguousarray(np.asarray(w, np.float32).T.astype(NP_MM_DT))
          for g, w in (("f", W_f), ("i", W_i), ("h", W_h))}
    bs = {g: np.asarray(b, np.float32) for g, b in (("f", b_f), ("i", b_i), ("h", b_h))}
    xTb = [np.ascontiguousarray(x[b].T.astype(NP_MM_DT)) for b in range(B)]

    maps = []
    for c in range(N_CORES):
        b, hh = divmod(c, 2)
        hsl = slice(hh * HSH, (hh + 1) * HSH)
        bias_pack = np.concatenate([
            bs["f"][hsl].reshape(IT, P).T,
            bs["i"][hsl].reshape(IT, P).T,
            bs["h"][hsl].reshape(IT, P).T,
            (bs["h"][hsl] + 0.5).reshape(IT, P).T,
        ], axis=1)
        maps.append({
            "xT": xTb[b],
            "wf": np.ascontiguousarray(wT["f"][:, hsl]),
            "wi": np.ascontiguousarray(wT["i"][:, hsl]),
            "wh": np.ascontiguousarray(wT["h"][:, hsl]),
            "biases": np.ascontiguousarray(bias_pack, dtype=np.float32),
        })
    return maps


def kernel(x, W_f, b_f, W_i, b_i, W_h, b_h):
    global _COMPILED
    if _COMPILED is None:
        _COMPILED = _build()
    nc = _COMPILED

    res = run_bass_kernel_spmd(
        nc, _in_maps(x, W_f, b_f, W_i, b_i, W_h, b_h), list(range(N_CORES)))

    full = np.empty((B, T, DH), np.float32)
    for c in range(N_CORES):
        b, hh = divmod(c, 2)
        full[b, :, hh * HSH:(hh + 1) * HSH] = res.results[c]["out"].T
    return full


# revision 20
# speedup vs baseline: 1.0460x; 1.0460x over previous
"""MinLSTM layer on 8 Trainium2 NeuronCores.

Math (equivalent to the log-space reference, done in linear space):
    f_pre = x @ W_f.T + b_f ; i_pre = x @ W_i.T + b_i ; h_pre = x @ W_h.T + b_h
    sf = sigmoid(f_pre) ; si = sigmoid(i_pre)
    f = sf / (sf + si)                       # normalized forget gate
    i = 1 - f                                # = si / (sf + si)
    g = max(sigmoid(h_pre), h_pre + 0.5)     # == exp(log_g), exactly
    h_t = f_t * h_{t-1} + i_t * g_t,  h_0 = 1
The gates satisfy f in (0,1), g > 0, so h stays in a tame range and the
recurrence is numerically stable in fp32. Matmul operands are bf16 (max rel
err vs the fp32 log-space reference ~7e-3, well under the 2e-2 gate); PSUM
accumulation and all post-matmul arithmetic are fp32.

Sharding: 8 cores = batch(4) x hidden-halves(2). Core c handles batch b=c//2,
hidden slice [(c%2)*512, (c%2+1)*512). No cross-core communication; the scan
runs along T inside each core via the DVE TensorTensorScan instruction
(state = f*state - mv per step, mv = (f-1)*g = -i*g).

Device layout: gates computed as [h_part, t_free] via out = W_sliceT.T @ xT;
host pre-transposes x and W (numpy, to bf16) and re-transposes the [512, 4096]
per-core output back to [T, Dh]. Matmuls run in 512-wide t-chunks (one PSUM
bank); elementwise+scan run in up-to-1024-wide super-chunks.

Scheduling notes:
- bf16 operands halve HBM traffic (x 8MB, W 3MB per core) so the DMA stream
  (~390 GB/s) always stays ahead of the PE's gate-major J0 burn rate
  (~300 GB/s) -- no mid-stream PE stall, no HAM re-throttle.
- Only 2 warmup matmuls: real matmuls start as soon as the first (x, W_f)
  k-slices land and run at the cold 1.2 GHz rate until the HAM window
  (~3.4us) elapses; running real work cold beats burning the window on
  scratch warmups.
- In J1+ units, the two 512-wide halves of a (gate, k) pair run back-to-back
  on the same loaded weight tile (walrus can skip the second LDWEIGHTS).
- Elementwise work is split across three engines so no single engine's queue
  sits on the critical path: ACT does the 3 sigmoids (PSUM->SBUF), Pool does
  s = sf+si and f = sf*r, DVE does g = max(h_pre+.5, sg) (PSUM read),
  r = 1/s, mv = (f-1)*g, and the scan.
- The final chunk's last h-tile runs its elementwise/scan at 256-wide grain
  to shorten the serial chain after the last matmul.
"""

import sys

for _p in ("/opt/trn_rl_repo",):
    if _p not in sys.path:
        sys.path.append(_p)

import numpy as np
from ml_dtypes import bfloat16

import concourse.bass as bass
import concourse.tile as tile
from concourse import bacc, mybir
from concourse.bass_utils import run_bass_kernel_spmd

B, T, DIN, DH = 4, 4096, 1024, 1024
N_CORES = 8
HSH = DH // 2          # 512 hidden channels per core
P = 128                # partitions
KT = DIN // P          # 8 contraction tiles
NT = 512               # matmul t-chunk (free dim, one PSUM bank)
IT = HSH // P          # 4 h-tiles per core
# elementwise/scan super-chunks (start, length); tail chunks smaller to
# shrink the end-of-kernel drain
CHUNKS = [(0, 1024), (1024, 1024), (2048, 1024), (3072, 512), (3584, 512)]

MM_DT = mybir.dt.bfloat16
NP_MM_DT = bfloat16

_COMPILED = None


def _build():
    AF = mybir.ActivationFunctionType
    OP = mybir.AluOpType
    f32 = mybir.dt.float32

    nc = bacc.Bacc("TRN2", target_bir_lowering=False, debug=False)

    xT = nc.dram_tensor("xT", [DIN, T], MM_DT, kind="ExternalInput").ap()
    wd = {g: nc.dram_tensor(f"w{g}", [DIN, HSH], MM_DT, kind="ExternalInput").ap()
          for g in ("f", "i", "h")}
    # packed per-partition scalars: [b_f | b_i | b_h | b_h+0.5], each (128, IT)
    biases = nc.dram_tensor("biases", [P, 4 * IT], f32, kind="ExternalInput").ap()
    out = nc.dram_tensor("out", [HSH, T], f32, kind="ExternalOutput").ap()

    # DRAM views: (KT*P, n) -> [p, k, n]
    xT_v = xT.rearrange("(k p) t -> p k t", p=P)
    w_v = {g: w.rearrange("(k p) h -> p k h", p=P) for g, w in wd.items()}

    with tile.TileContext(nc) as tc:
        with (
            tc.tile_pool(name="wpool", bufs=1) as wpool,
            tc.tile_pool(name="bpool", bufs=1) as bpool,
            tc.tile_pool(name="xpool", bufs=32) as xpool,
            tc.tile_pool(name="psum", bufs=8, space="PSUM") as pspool,
            tc.tile_pool(name="work", bufs=4) as work,
            tc.tile_pool(name="hpool", bufs=6) as hpool,
        ):
            bias_t = bpool.tile([P, 4 * IT], f32, tag="bias")

            # per-k weight tiles, resident all kernel
            wt = {g: [wpool.tile([P, HSH], MM_DT, tag=f"w{g}{k}", name=f"w{g}{k}_t")
                      for k in range(KT)] for g in ("f", "i", "h")}

            def dma_w(g):
                for k in range(KT):
                    nc.sync.dma_start(out=wt[g][k][:], in_=w_v[g][:, k, :])

            def x_ktiles(t0, nt):
                """One [P, nt] tile per contraction slice k of a super-chunk.

                nt covers the whole super-chunk (up to 1024 cols = 2KB
                per-partition lines): one DMA per k-slice, matmuls take
                512-wide column slices of the tile.
                """
                xs = []
                for k in range(KT):
                    xk = xpool.tile([P, nt], MM_DT, tag="xk", name="xk_t")
                    nc.sync.dma_start(out=xk[:], in_=xT_v[:, k, t0:t0 + nt])
                    xs.append(xk)
                return xs

            def bias_ap(kind, i):
                return bias_t[:, kind * IT + i:kind * IT + i + 1]

            def chain(i, sf, si, gt, J, t0, ne, grain=None, pool_ok=True):
                """Normalize gates, build -i*g, scan, and store chunk.

                sf/si are the raw sigmoids; gt already holds g (from the
                per-half PSUM stage). grain < ne splits the work into
                sub-chunks so the last chunk's serial chain is short.
                (pool_ok is unused: the Pool engine measured 2x slower per
                op and its chain latency cost more than the DVE decongestion
                won; DVE TensorTensor divide is rejected by codegen, so the
                normalization stays as add + reciprocal + mul on the DVE.)
                """
                grain = grain or ne
                for c0 in range(0, ne, grain):
                    cs = slice(c0, c0 + grain)
                    nc.vector.tensor_add(si[:, cs], sf[:, cs], si[:, cs])
                    r = work.tile([P, grain], f32, tag="s", name="r_t")
                    nc.vector.reciprocal_approx_fast(out=r[:], in_=si[:, cs])
                    nc.vector.tensor_mul(sf[:, cs], sf[:, cs], r[:])      # f
                    nc.vector.scalar_tensor_tensor(                # mv=(f-1)*g
                        gt[:, cs], sf[:, cs], 1.0, gt[:, cs],
                        op0=OP.subtract, op1=OP.mult)
                    hc = hpool.tile([P, grain], f32, tag="h", name=f"h{i}_t")
                    init = 1.0 if J == 0 and c0 == 0 else hprev[i][:, -1:]
                    nc.vector.tensor_tensor_scan(
                        hc[:], sf[:, cs], gt[:, cs], init,
                        op0=OP.mult, op1=OP.subtract)
                    hprev[i] = hc
                    nc.sync.dma_start(
                        out=out[i * P:(i + 1) * P, t0 + c0:t0 + c0 + grain],
                        in_=hc[:])

            hprev = [None] * IT
            hsls = [slice(i * P, (i + 1) * P) for i in range(IT)]

            # Warmup matmuls on a zeroed scratch tile open the PE's HAM
            # activity window while the first (x, W_f) DMAs land; the memset
            # runs on the Pool queue, whose preamble finishes first, so the
            # PE goes busy the moment its own preamble ends. Six warmups
            # bridge until real data arrives (~8.3us) without a gap that
            # would reset the HAM window; the clock then reaches 2.4 GHz
            # ~3.4us after the first warmup instead of mid-stream.
            scratch = bpool.tile([P, NT], MM_DT, tag="scratch")
            nc.gpsimd.memset(scratch[:].bitcast(mybir.dt.uint16), 0)
            pswarm = pspool.tile([P, NT], f32, tag="ps", name="pswarm_t")
            for _ in range(6):
                nc.tensor.matmul(pswarm[:], lhsT=scratch[:, :P], rhs=scratch[:],
                                 start=True, stop=True)

            # ---- J0: gate-major, k-outer; PE chases the input DMA stream ----
            t0, ne = CHUNKS[0]
            nhalf = ne // NT
            # priority order: (x_h0[k], W_f[k]) pairs, x_h1, W_i, W_h
            xcs = [[xpool.tile([P, NT], MM_DT, tag="xk", name="xk_t")
                    for _ in range(KT)] for _ in range(nhalf)]
            for k in range(KT):
                nc.sync.dma_start(out=xcs[0][k][:], in_=xT_v[:, k, t0:t0 + NT])
                nc.sync.dma_start(out=wt["f"][k][:], in_=w_v["f"][:, k, :])
                if k == 0:
                    # bias is tiny and first needed by the ACTs at ~12us;
                    # issue it after the first matmul's dependencies
                    nc.sync.dma_start(out=bias_t[:], in_=biases[:])
            for h in range(1, nhalf):
                th = t0 + h * NT
                for k in range(KT):
                    nc.sync.dma_start(out=xcs[h][k][:], in_=xT_v[:, k, th:th + NT])
            dma_w("i")
            dma_w("h")

            sf = [work.tile([P, ne], f32, tag="sf", name="sf_t") for _ in range(IT)]
            si = [work.tile([P, ne], f32, tag="si", name="si_t") for _ in range(IT)]
            sg = [work.tile([P, ne], f32, tag="sg", name="sg_t") for _ in range(IT)]
            gt = [work.tile([P, ne], f32, tag="gt", name="gt_t") for _ in range(IT)]
            for gate, dsts, bk in (("f", sf, 0), ("i", si, 1), ("h", sg, 2)):
                for half in range(nhalf):
                    esl = slice(half * NT, (half + 1) * NT)
                    psts = [pspool.tile([P, NT], f32, tag="ps", name="ps_t")
                            for _ in range(IT)]
                    for k in range(KT):
                        for pst, hsl in zip(psts, hsls):
                            nc.tensor.matmul(
                                pst[:], lhsT=wt[gate][k][:, hsl],
                                rhs=xcs[half][k][:],
                                start=(k == 0), stop=(k == KT - 1))
                    for i in range(IT):
                        nc.scalar.activation(dsts[i][:, esl], psts[i][:], AF.Sigmoid,
                                             bias=bias_ap(bk, i), scale=1.0)
                        if gate == "h":
                            nc.vector.scalar_tensor_tensor(
                                gt[i][:, esl], psts[i][:], bias_ap(3, i),
                                sg[i][:, esl], op0=OP.add, op1=OP.max)
            for i in range(IT):
                chain(i, sf[i], si[i], gt[i], 0, t0, ne)

            # ---- J1+: h-tile-major units; strips share each weight tile ----
            for J, (t0, ne) in enumerate(CHUNKS[1:], start=1):
                st = min(NT, ne)            # matmul strip width
                nstrip = ne // st
                xks = x_ktiles(t0, ne)
                xcs = [[xk[:, h * st:(h + 1) * st] for xk in xks]
                       for h in range(nstrip)]
                for i in range(IT):
                    hsl = hsls[i]
                    sf = work.tile([P, ne], f32, tag="sf", name="sf_t")
                    si = work.tile([P, ne], f32, tag="si", name="si_t")
                    sg = work.tile([P, ne], f32, tag="sg", name="sg_t")
                    gt = work.tile([P, ne], f32, tag="gt", name="gt_t")
                    for gate, dst, bk in (("f", sf, 0), ("i", si, 1),
                                          ("h", sg, 2)):
                        psts = [pspool.tile([P, st], f32, tag="ps", name="ps_t")
                                for _ in range(nstrip)]
                        for k in range(KT):
                            for half in range(nstrip):
                                nc.tensor.matmul(
                                    psts[half][:], lhsT=wt[gate][k][:, hsl],
                                    rhs=xcs[half][k],
                                    start=(k == 0), stop=(k == KT - 1))
                        for half in range(nstrip):
                            esl = slice(half * st, (half + 1) * st)
                            nc.scalar.activation(dst[:, esl], psts[half][:],
                                                 AF.Sigmoid, bias=bias_ap(bk, i),
                                                 scale=1.0)
                            if gate == "h":
                                nc.vector.scalar_tensor_tensor(
                                    gt[:, esl], psts[half][:], bias_ap(3, i),
                                    sg[:, esl], op0=OP.add, op1=OP.max)
                    chain(i, sf, si, gt, J, t0, ne,
                          grain=256 if (J == len(CHUNKS) - 1 and i == IT - 1)
                          else None)

    nc.compile()
    return nc


def _in_maps(x, W_f, b_f, W_i, b_i, W_h, b_h):
    x = np.asarray(x, np.float32)
    wT = {g: np.ascontiguousarray(np.asarray(w, np.float32).T.astype(NP_MM_DT))
          for g, w in (("f", W_f), ("i", W_i), ("h", W_h))}
    bs = {g: np.asarray(b, np.float32) for g, b in (("f", b_f), ("i", b_i), ("h", b_h))}
    xTb = [np.ascontiguousarray(x[b].T.astype(NP_MM_DT)) for b in range(B)]

    maps = []
    for c in range(N_CORES):
        b, hh = divmod(c, 2)
        hsl = slice(hh * HSH, (hh + 1) * HSH)
        bias_pack = np.concatenate([
            bs["f"][hsl].reshape(IT, P).T,
            bs["i"][hsl].reshape(IT, P).T,
            bs["h"][hsl].reshape(IT, P).T,
            (bs["h"][hsl] + 0.5).reshape(IT, P).T,
        ], axis=1)
        maps.append({
            "xT": xTb[b],
            "wf": np.ascontiguousarray(wT["f"][:, hsl]),
            "wi": np.ascontiguousarray(wT["i"][:, hsl]),
            "wh": np.ascontiguousarray(wT["h"][:, hsl]),
            "biases": np.ascontiguousarray(bias_pack, dtype=np.float32),
        })
    return maps


def kernel(x, W_f, b_f, W_i, b_i, W_h, b_h):
    global _COMPILED
    if _COMPILED is None:
        _COMPILED = _build()
    nc = _COMPILED

    res = run_bass_kernel_spmd(
        nc, _in_maps(x, W_f, b_f, W_i, b_i, W_h, b_h), list(range(N_CORES)))

    full = np.empty((B, T, DH), np.float32)
    for c in range(N_CORES):
        b, hh = divmod(c, 2)
        full[b, :, hh * HSH:(hh + 1) * HSH] = res.results[c]["out"].T
    return full
